# revision 4
# baseline (speedup 1.0000x reference)
"""Trainium2 Bass kernel for attention pooling:
    v = tanh(x @ W + b) / sqrt(A);  vu = v @ u;  alphas = softmax(vu) (+1e-10 in denom)
    out = sum_s alphas[s] * x[s]
Shapes: x [B=128, S=2048, H=512], W [512, 512], b/u [512].
Returns (out [B, H], alphas [B, S]).

Strategy: data-parallel over 8 NeuronCores (16 batch rows each). Per row:
  - cast-DMA x row fp32->bf16 into SBUF. s is tiled partition-major
    (partition p holds rows s = 16p+t, t=0..15) so each partition reads ONE
    contiguous 32KB DRAM span -> minimal SWDGE descriptors, full HBM rate.
  - one xbar DMA transpose -> x^T chunks for the PE
  - 64 bf16 matmuls (K=512 contraction) -> PSUM; ACT fused tanh PSUM->SBUF
  - DVE scalar_tensor_tensor: vu = sum_a v*(u/sqrt(A)) fused mul+reduce
  - softmax without max-subtraction (vu is bounded, exp safe):
      e = exp(vu); denom = sum(e) + 1e-10; out = (sum_s e_s x_s)/denom; alphas = e/denom
    (algebraically identical to the reference's max-subtracted form)
  - weighted sum via M=1 PE matmuls accumulated in PSUM.
"""
import functools
from contextlib import ExitStack

import numpy as np

import concourse.bacc as bacc
import concourse.tile as tile
from concourse import mybir
from concourse.bass_utils import run_bass_kernel_spmd

F32 = mybir.dt.float32
BF16 = mybir.dt.bfloat16
AF = mybir.ActivationFunctionType
ALU = mybir.AluOpType

B, S, H, A = 128, 2048, 512, 512
N_CORES = 8
R = B // N_CORES          # batch rows per core
P = 128                   # partitions
NT = S // P               # s-tiles per row
KC = H // P               # k-chunks in the contraction


def build(n_rows=R, s_len=S, has_bias=False):
    """Build + compile the per-core Bass program."""
    nt = s_len // P
    nc = bacc.Bacc("TRN2", target_bir_lowering=False, debug=False)

    x_d = nc.dram_tensor("x", [n_rows, s_len, H], F32, kind="ExternalInput")
    w_d = nc.dram_tensor("w", [H, A], F32, kind="ExternalInput")
    u_d = nc.dram_tensor("u", [A], F32, kind="ExternalInput")
    if has_bias:
        b_d = nc.dram_tensor("b", [A], F32, kind="ExternalInput")
    out_d = nc.dram_tensor("out", [n_rows, H], F32, kind="ExternalOutput")
    al_d = nc.dram_tensor("alphas", [n_rows, s_len], F32, kind="ExternalOutput")

    inv_sqrt_a = 1.0 / float(np.sqrt(A))

    with tile.TileContext(nc) as tc, ExitStack() as ctx:
        consts = ctx.enter_context(tc.tile_pool(name="consts", bufs=1))
        xp = ctx.enter_context(tc.tile_pool(name="xp", bufs=4))
        xtp = ctx.enter_context(tc.tile_pool(name="xtp", bufs=3))
        vp = ctx.enter_context(tc.tile_pool(name="vp", bufs=4))
        scr = ctx.enter_context(tc.tile_pool(name="scr", bufs=3))
        smp = ctx.enter_context(tc.tile_pool(name="smp", bufs=2))
        pp = ctx.enter_context(tc.tile_pool(name="pp", bufs=4, space="PSUM"))
        wp = ctx.enter_context(tc.tile_pool(name="wp", bufs=2, space="PSUM"))
        tailp = ctx.enter_context(tc.tile_pool(name="tailp", bufs=2, space="PSUM"))

        # ---- constants
        ones_row_bf = consts.tile([1, P], BF16, tag="ones_bf")
        nc.gpsimd.memset(ones_row_bf[:], 1.0)
        ones_row_f = consts.tile([1, P], F32, tag="ones_f")
        nc.gpsimd.memset(ones_row_f[:], 1.0)
        ones_col_f = nc.const_aps.tensor(1.0, (P, 1), F32)

        # W (bf16, 4 chunks of [128, A]) via cast-DMA
        w_sb = []
        for k in range(KC):
            wk = consts.tile([P, A], BF16, tag=f"w{k}")
            nc.gpsimd.dma_start(wk[:], w_d.ap()[k * P:(k + 1) * P, :])
            w_sb.append(wk)

        # u/sqrt(A) broadcast to all partitions (K=1 ones-matmul), bf16
        u_row = consts.tile([1, A], BF16, tag="urow")
        nc.gpsimd.dma_start(u_row[:], u_d.ap().rearrange("(o h) -> o h", o=1))
        ub_ps = tailp.tile([P, A], F32, tag="tail")
        nc.tensor.matmul(ub_ps[:, :], ones_row_bf[:], u_row[:], start=True, stop=True)
        u_b = consts.tile([P, A], BF16, tag="ub")
        nc.scalar.activation(u_b[:], ub_ps[:, :], AF.Copy, scale=inv_sqrt_a)

        if has_bias:
            b_row = consts.tile([1, A], BF16, tag="brow")
            nc.gpsimd.dma_start(b_row[:], b_d.ap().rearrange("(o h) -> o h", o=1))

        for r in range(n_rows):
            # ---- load row (cast fp32->bf16), partition-major s tiling:
            # X[p, t*H:(t+1)*H] = x[r, nt*p + t, :]  -> one contiguous DRAM
            # span per partition (nt*H floats)
            X = xp.tile([P, nt * H], BF16, tag="X")
            nc.gpsimd.dma_start(
                X[:].rearrange("p (t h) -> p t h", h=H),
                x_d.ap()[r].rearrange("(p t) h -> p t h", p=P),
            )
            # ---- transpose whole row: chunk c = t*KC + k holds tile_t[:, k*128:...].T
            XT = xtp.tile([P, nt * KC, P], BF16, tag="XT")
            nc.sync.dma_start_transpose(XT[:], X[:])

            vu_row = smp.tile([P, nt], F32, tag="vu")

            for t in range(nt):
                p_ps = pp.tile([P, A], F32, tag="p")
                if has_bias:
                    nc.tensor.matmul(p_ps[:, :], ones_row_bf[:], b_row[:],
                                     start=True, stop=False)
                for k in range(KC):
                    nc.tensor.matmul(
                        p_ps[:, :],
                        XT[:, t * KC + k, :],
                        w_sb[k][:],
                        start=(k == 0 and not has_bias),
                        stop=(k == KC - 1),
                    )
                v = vp.tile([P, A], BF16, tag="v")
                nc.scalar.activation(v[:], p_ps[:, :], AF.Tanh)
                sc = scr.tile([P, A], BF16, tag="sc")
                nc.vector.scalar_tensor_tensor(
                    out=sc[:], in0=v[:], scalar=1.0, in1=u_b[:],
                    op0=ALU.mult, op1=ALU.mult,
                    accum_out=vu_row[:, t:t + 1],
                )

            # ---- e = exp(vu): fp32 for alphas, bf16 for the weighted sum
            e_f = smp.tile([P, nt], F32, tag="ef")
            sums = smp.tile([P, 1], F32, tag="sums")
            nc.scalar.activation(e_f[:], vu_row[:], AF.Exp, accum_out=sums[:])
            e_bf = smp.tile([P, nt], BF16, tag="ebf")
            nc.vector.tensor_copy(e_bf[:], e_f[:])

            # ---- weighted sum: wsum[1, H] = sum_t e[:, t].T @ X_t
            wsum_ps = wp.tile([1, H], F32, tag="w")
            for t in range(nt):
                nc.tensor.matmul(
                    wsum_ps[:, :],
                    e_bf[:, t:t + 1],
                    X[:, t * H:(t + 1) * H],
                    start=(t == 0), stop=(t == nt - 1),
                )

            # ---- denom = sum(e) + 1e-10; rinv = 1/denom
            ssum_ps = tailp.tile([1, 1], F32, tag="tail")
            nc.tensor.matmul(ssum_ps[:, :], sums[:], ones_col_f, start=True, stop=True)
            denom = smp.tile([1, 1], F32, tag="denom")
            nc.vector.tensor_scalar_add(denom[:], ssum_ps[:, :], 1e-10)
            rinv = smp.tile([1, 1], F32, tag="rinv")
            nc.vector.reciprocal(rinv[:], denom[:])

            # ---- out row = wsum * rinv
            out_sb = smp.tile([1, H], F32, tag="outsb")
            nc.scalar.activation(out_sb[:], wsum_ps[:, :], AF.Copy, scale=rinv[:, :])
            nc.scalar.dma_start(out_d.ap()[r].rearrange("(o h) -> o h", o=1), out_sb[:])

            # ---- alphas row = e_f * rinv; element (p, t) -> s = nt*p + t
            bc_ps = tailp.tile([P, 1], F32, tag="tail")
            nc.tensor.matmul(bc_ps[:, :], ones_row_f[:], rinv[:], start=True, stop=True)
            r_b = smp.tile([P, 1], F32, tag="rb")
            nc.scalar.copy(r_b[:], bc_ps[:, :])
            a_sc = smp.tile([P, nt], F32, tag="asc")
            nc.vector.tensor_scalar_mul(a_sc[:], e_f[:], r_b[:])
            nc.scalar.dma_start(
                al_d.ap()[r].rearrange("(p t) -> p t", p=P), a_sc[:],
            )

    nc.compile()
    return nc


@functools.lru_cache(maxsize=2)
def _built(has_bias: bool):
    return build(R, S, has_bias)


def kernel(lstm_output, w_omega, b_omega, u_omega):
    lstm_output = np.ascontiguousarray(np.asarray(lstm_output, dtype=np.float32))
    w_omega = np.ascontiguousarray(np.asarray(w_omega, dtype=np.float32))
    b_omega = np.ascontiguousarray(np.asarray(b_omega, dtype=np.float32))
    u_omega = np.ascontiguousarray(np.asarray(u_omega, dtype=np.float32))

    has_bias = bool(np.any(b_omega != 0.0))
    nc = _built(has_bias)

    in_maps = []
    for c in range(N_CORES):
        m = {
            "x": lstm_output[c * R:(c + 1) * R],
            "w": w_omega,
            "u": u_omega,
        }
        if has_bias:
            m["b"] = b_omega
        in_maps.append(m)

    res = run_bass_kernel_spmd(nc, in_maps, core_ids=list(range(N_CORES)))
    out = np.concatenate([r["out"] for r in res.results], axis=0)
    alphas = np.concatenate([r["alphas"] for r in res.results], axis=0)
    return out, alphas


# revision 6
# speedup vs baseline: 1.1340x; 1.1340x over previous
"""Trainium2 Bass kernel for attention pooling:
    v = tanh(x @ W + b) / sqrt(A);  vu = v @ u;  alphas = softmax(vu) (+1e-10 in denom)
    out = sum_s alphas[s] * x[s]
Shapes: x [B=128, S=2048, H=512], W [512, 512], b/u [512].
Returns (out [B, H], alphas [B, S]).

Strategy: data-parallel over 8 NeuronCores (16 batch rows each). Per row:
  - cast-DMA x row fp32->bf16 into SBUF. s is tiled partition-major
    (partition p holds rows s = 16p+t, t=0..15) so each partition reads ONE
    contiguous 32KB DRAM span -> minimal SWDGE descriptors, full HBM rate.
  - one xbar DMA transpose -> x^T chunks for the PE
  - 64 bf16 matmuls (K=512 contraction) -> PSUM; ACT fused tanh PSUM->SBUF
  - DVE scalar_tensor_tensor: vu = sum_a v*(u/sqrt(A)) fused mul+reduce
  - softmax without max-subtraction (vu is bounded, exp safe):
      e = exp(vu); denom = sum(e) + 1e-10; out = (sum_s e_s x_s)/denom; alphas = e/denom
    (algebraically identical to the reference's max-subtracted form)
  - weighted sum via M=1 PE matmuls accumulated in PSUM.
"""
import functools
from contextlib import ExitStack

import numpy as np

import concourse.bacc as bacc
import concourse.tile as tile
from concourse import mybir
from concourse.bass_utils import run_bass_kernel_spmd

F32 = mybir.dt.float32
BF16 = mybir.dt.bfloat16
AF = mybir.ActivationFunctionType
ALU = mybir.AluOpType

B, S, H, A = 128, 2048, 512, 512
N_CORES = 8
R = B // N_CORES          # batch rows per core
P = 128                   # partitions
NT = S // P               # s-tiles per row
KC = H // P               # k-chunks in the contraction


def build(n_rows=R, s_len=S, has_bias=False):
    """Build + compile the per-core Bass program."""
    nt = s_len // P
    nc = bacc.Bacc("TRN2", target_bir_lowering=False, debug=False)

    x_d = nc.dram_tensor("x", [n_rows, s_len, H], F32, kind="ExternalInput")
    w_d = nc.dram_tensor("w", [H, A], F32, kind="ExternalInput")
    u_d = nc.dram_tensor("u", [A], F32, kind="ExternalInput")
    if has_bias:
        b_d = nc.dram_tensor("b", [A], F32, kind="ExternalInput")
    out_d = nc.dram_tensor("out", [n_rows, H], F32, kind="ExternalOutput")
    al_d = nc.dram_tensor("alphas", [n_rows, s_len], F32, kind="ExternalOutput")

    inv_sqrt_a = 1.0 / float(np.sqrt(A))

    with tile.TileContext(nc) as tc, ExitStack() as ctx:
        consts = ctx.enter_context(tc.tile_pool(name="consts", bufs=1))
        xp = ctx.enter_context(tc.tile_pool(name="xp", bufs=4))
        xtp = ctx.enter_context(tc.tile_pool(name="xtp", bufs=3))
        vp = ctx.enter_context(tc.tile_pool(name="vp", bufs=4))
        scr = ctx.enter_context(tc.tile_pool(name="scr", bufs=3))
        smp = ctx.enter_context(tc.tile_pool(name="smp", bufs=2))
        pp = ctx.enter_context(tc.tile_pool(name="pp", bufs=4, space="PSUM"))
        wp = ctx.enter_context(tc.tile_pool(name="wp", bufs=2, space="PSUM"))
        tailp = ctx.enter_context(tc.tile_pool(name="tailp", bufs=2, space="PSUM"))

        # ---- constants
        ones_row_bf = consts.tile([1, P], BF16, tag="ones_bf")
        nc.gpsimd.memset(ones_row_bf[:], 1.0)
        ones_row_f = consts.tile([1, P], F32, tag="ones_f")
        nc.gpsimd.memset(ones_row_f[:], 1.0)
        ones_col_f = nc.const_aps.tensor(1.0, (P, 1), F32)

        # W (bf16, 4 chunks of [128, A]) via cast-DMA
        w_sb = []
        for k in range(KC):
            wk = consts.tile([P, A], BF16, tag=f"w{k}")
            nc.gpsimd.dma_start(wk[:], w_d.ap()[k * P:(k + 1) * P, :])
            w_sb.append(wk)

        # u/sqrt(A) broadcast to all partitions (K=1 ones-matmul), bf16
        u_row = consts.tile([1, A], BF16, tag="urow")
        nc.gpsimd.dma_start(u_row[:], u_d.ap().rearrange("(o h) -> o h", o=1))
        ub_ps = tailp.tile([P, A], F32, tag="tail")
        nc.tensor.matmul(ub_ps[:, :], ones_row_bf[:], u_row[:], start=True, stop=True)
        u_b = consts.tile([P, A], BF16, tag="ub")
        nc.scalar.activation(u_b[:], ub_ps[:, :], AF.Copy, scale=inv_sqrt_a)

        if has_bias:
            b_row = consts.tile([1, A], BF16, tag="brow")
            nc.gpsimd.dma_start(b_row[:], b_d.ap().rearrange("(o h) -> o h", o=1))

        for r in range(n_rows):
            # ---- load row (cast fp32->bf16), partition-major s tiling:
            # X[p, t*H:(t+1)*H] = x[r, nt*p + t, :]  -> one contiguous DRAM
            # span per partition (nt*H floats)
            X = xp.tile([P, nt * H], BF16, tag="X")
            nc.gpsimd.dma_start(
                X[:].rearrange("p (t h) -> p t h", h=H),
                x_d.ap()[r].rearrange("(p t) h -> p t h", p=P),
            )
            # ---- transpose whole row: chunk c = t*KC + k holds tile_t[:, k*128:...].T
            XT = xtp.tile([P, nt * KC, P], BF16, tag="XT")
            nc.sync.dma_start_transpose(XT[:], X[:])

            vu_row = smp.tile([P, nt], F32, tag="vu")

            for t in range(nt):
                p_ps = pp.tile([P, A], F32, tag="p")
                if has_bias:
                    nc.tensor.matmul(p_ps[:, :], ones_row_bf[:], b_row[:],
                                     start=True, stop=False)
                for k in range(KC):
                    nc.tensor.matmul(
                        p_ps[:, :],
                        XT[:, t * KC + k, :],
                        w_sb[k][:],
                        start=(k == 0 and not has_bias),
                        stop=(k == KC - 1),
                    )
                v = vp.tile([P, A], BF16, tag="v")
                nc.scalar.activation(v[:], p_ps[:, :], AF.Tanh)
                sc = scr.tile([P, A], BF16, tag="sc")
                nc.vector.scalar_tensor_tensor(
                    out=sc[:], in0=v[:], scalar=1.0, in1=u_b[:],
                    op0=ALU.mult, op1=ALU.mult,
                    accum_out=vu_row[:, t:t + 1],
                )

            # ---- e = exp(vu): fp32 for alphas, bf16 for the weighted sum
            e_f = smp.tile([P, nt], F32, tag="ef")
            sums = smp.tile([P, 1], F32, tag="sums")
            nc.scalar.activation(e_f[:], vu_row[:], AF.Exp, accum_out=sums[:])
            e_bf = smp.tile([P, nt], BF16, tag="ebf")
            nc.vector.tensor_copy(e_bf[:], e_f[:])

            # ---- weighted sum: wsum[1, H] = sum_t e[:, t].T @ X_t
            wsum_ps = wp.tile([1, H], F32, tag="w")
            for t in range(nt):
                nc.tensor.matmul(
                    wsum_ps[:, :],
                    e_bf[:, t:t + 1],
                    X[:, t * H:(t + 1) * H],
                    start=(t == 0), stop=(t == nt - 1),
                )

            # ---- denom = sum(e) + 1e-10; rinv = 1/denom
            ssum_ps = tailp.tile([1, 1], F32, tag="tail")
            nc.tensor.matmul(ssum_ps[:, :], sums[:], ones_col_f, start=True, stop=True)
            denom = smp.tile([1, 1], F32, tag="denom")
            nc.vector.tensor_scalar_add(denom[:], ssum_ps[:, :], 1e-10)
            rinv = smp.tile([1, 1], F32, tag="rinv")
            nc.vector.reciprocal(rinv[:], denom[:])

            # ---- out row = wsum * rinv
            out_sb = smp.tile([1, H], F32, tag="outsb")
            nc.scalar.activation(out_sb[:], wsum_ps[:, :], AF.Copy, scale=rinv[:, :])
            nc.scalar.dma_start(out_d.ap()[r].rearrange("(o h) -> o h", o=1), out_sb[:])

            # ---- alphas row = e_f * rinv; element (p, t) -> s = nt*p + t
            bc_ps = tailp.tile([P, 1], F32, tag="tail")
            nc.tensor.matmul(bc_ps[:, :], ones_row_f[:], rinv[:], start=True, stop=True)
            r_b = smp.tile([P, 1], F32, tag="rb")
            nc.scalar.copy(r_b[:], bc_ps[:, :])
            a_sc = smp.tile([P, nt], F32, tag="asc")
            nc.vector.tensor_scalar_mul(a_sc[:], e_f[:], r_b[:])
            nc.scalar.dma_start(
                al_d.ap()[r].rearrange("(p t) -> p t", p=P), a_sc[:],
            )

    nc.compile()
    return nc


@functools.lru_cache(maxsize=2)
def _built(has_bias: bool):
    return build(R, S, has_bias)


def kernel(lstm_output, w_omega, b_omega, u_omega):
    lstm_output = np.ascontiguousarray(np.asarray(lstm_output, dtype=np.float32))
    w_omega = np.ascontiguousarray(np.asarray(w_omega, dtype=np.float32))
    b_omega = np.ascontiguousarray(np.asarray(b_omega, dtype=np.float32))
    u_omega = np.ascontiguousarray(np.asarray(u_omega, dtype=np.float32))

    has_bias = bool(np.any(b_omega != 0.0))
    nc = _built(has_bias)

    in_maps = []
    for c in range(N_CORES):
        m = {
            "x": lstm_output[c * R:(c + 1) * R],
            "w": w_omega,
            "u": u_omega,
        }
        if has_bias:
            m["b"] = b_omega
        in_maps.append(m)

    res = run_bass_kernel_spmd(nc, in_maps, core_ids=list(range(N_CORES)))
    out = np.concatenate([r["out"] for r in res.results], axis=0)
    alphas = np.concatenate([r["alphas"] for r in res.results], axis=0)
    return out, alphas


# revision 9
# speedup vs baseline: 1.2732x; 1.1228x over previous
"""Trainium2 Bass kernel for attention pooling:
    v = tanh(x @ W + b) / sqrt(A);  vu = v @ u;  alphas = softmax(vu) (+1e-10 in denom)
    out = sum_s alphas[s] * x[s]
Shapes: x [B=128, S=2048, H=512], W [512, 512], b/u [512].
Returns (out [B, H], alphas [B, S]).

Strategy: data-parallel over 8 NeuronCores (16 batch rows each). Per row:
  - cast-DMA x row fp32->bf16 into SBUF. s is tiled partition-major
    (partition p holds rows s = 16p+t, t=0..15) so each partition reads ONE
    contiguous 32KB DRAM span -> minimal SWDGE descriptors, full HBM rate.
  - one xbar DMA transpose -> x^T chunks for the PE
  - 64 bf16 matmuls (K=512 contraction) -> PSUM; ACT fused tanh PSUM->SBUF
  - DVE scalar_tensor_tensor: vu = sum_a v*(u/sqrt(A)) fused mul+reduce
  - softmax without max-subtraction (vu is bounded, exp safe):
      e = exp(vu); denom = sum(e) + 1e-10; out = (sum_s e_s x_s)/denom; alphas = e/denom
    (algebraically identical to the reference's max-subtracted form)
  - weighted sum via M=1 PE matmuls accumulated in PSUM.
"""
import functools
from contextlib import ExitStack

import numpy as np

import concourse.bacc as bacc
import concourse.tile as tile
from concourse import mybir
from concourse.bass_utils import run_bass_kernel_spmd

F32 = mybir.dt.float32
BF16 = mybir.dt.bfloat16
AF = mybir.ActivationFunctionType
ALU = mybir.AluOpType

B, S, H, A = 128, 2048, 512, 512
N_CORES = 8
R = B // N_CORES          # batch rows per core
P = 128                   # partitions
NT = S // P               # s-tiles per row
KC = H // P               # k-chunks in the contraction


def build(n_rows=R, s_len=S, has_bias=False):
    """Build + compile the per-core Bass program."""
    nt = s_len // P
    nc = bacc.Bacc("TRN2", target_bir_lowering=False, debug=False)

    x_d = nc.dram_tensor("x", [n_rows, s_len, H], F32, kind="ExternalInput")
    w_d = nc.dram_tensor("w", [H, A], F32, kind="ExternalInput")
    u_d = nc.dram_tensor("u", [A], F32, kind="ExternalInput")
    if has_bias:
        b_d = nc.dram_tensor("b", [A], F32, kind="ExternalInput")
    out_d = nc.dram_tensor("out", [n_rows, H], F32, kind="ExternalOutput")
    al_d = nc.dram_tensor("alphas", [n_rows, s_len], F32, kind="ExternalOutput")

    inv_sqrt_a = 1.0 / float(np.sqrt(A))

    with tile.TileContext(nc) as tc, ExitStack() as ctx:
        consts = ctx.enter_context(tc.tile_pool(name="consts", bufs=1))
        xp = ctx.enter_context(tc.tile_pool(name="xp", bufs=7))
        xtp = ctx.enter_context(tc.tile_pool(name="xtp", bufs=4))
        vp = ctx.enter_context(tc.tile_pool(name="vp", bufs=4))
        scr = ctx.enter_context(tc.tile_pool(name="scr", bufs=3))
        smp = ctx.enter_context(tc.tile_pool(name="smp", bufs=2))
        pp = ctx.enter_context(tc.tile_pool(name="pp", bufs=4, space="PSUM"))
        wp = ctx.enter_context(tc.tile_pool(name="wp", bufs=2, space="PSUM"))
        tailp = ctx.enter_context(tc.tile_pool(name="tailp", bufs=2, space="PSUM"))

        # ---- constants
        ones_row_bf = consts.tile([1, P], BF16, tag="ones_bf")
        nc.gpsimd.memset(ones_row_bf[:], 1.0)
        ones_row_f = consts.tile([1, P], F32, tag="ones_f")
        nc.gpsimd.memset(ones_row_f[:], 1.0)
        ones_col_f = nc.const_aps.tensor(1.0, (P, 1), F32)

        # W (bf16, 4 chunks of [128, A]) via cast-DMA
        w_sb = []
        for k in range(KC):
            wk = consts.tile([P, A], BF16, tag=f"w{k}")
            nc.gpsimd.dma_start(wk[:], w_d.ap()[k * P:(k + 1) * P, :])
            w_sb.append(wk)

        # u/sqrt(A) broadcast to all partitions (K=1 ones-matmul), bf16
        u_row = consts.tile([1, A], BF16, tag="urow")
        nc.gpsimd.dma_start(u_row[:], u_d.ap().rearrange("(o h) -> o h", o=1))
        ub_ps = tailp.tile([P, A], F32, tag="tail")
        nc.tensor.matmul(ub_ps[:, :], ones_row_bf[:], u_row[:], start=True, stop=True)
        u_b = consts.tile([P, A], BF16, tag="ub")
        nc.scalar.activation(u_b[:], ub_ps[:, :], AF.Copy, scale=inv_sqrt_a)

        if has_bias:
            b_row = consts.tile([1, A], BF16, tag="brow")
            nc.gpsimd.dma_start(b_row[:], b_d.ap().rearrange("(o h) -> o h", o=1))

        NH = 2                  # half-row DMA pipeline granularity
        nth = nt // NH
        for r in range(n_rows):
            vu_row = smp.tile([P, nt], F32, tag="vu")
            Xh_tiles = []
            for hf in range(NH):
                # ---- load half-row (cast fp32->bf16), partition-major:
                # Xh[p, t'*H:(t'+1)*H] = x[r, nt*p + hf*nth + t', :]
                # -> one contiguous DRAM span per partition
                Xh = xp.tile([P, nth * H], BF16, tag="X", name=f"X_{r}_{hf}")
                nc.gpsimd.dma_start(
                    Xh[:].rearrange("p (t h) -> p t h", h=H),
                    x_d.ap()[r].rearrange("(p t) h -> p t h", p=P)
                        [:, hf * nth:(hf + 1) * nth, :],
                )
                Xh_tiles.append(Xh)
                # ---- transpose half: chunk c = t'*KC + k
                XT = xtp.tile([P, nth * KC, P], BF16, tag="XT",
                              name=f"XT_{r}_{hf}")
                nc.sync.dma_start_transpose(XT[:], Xh[:])

                for tt in range(nth):
                    t = hf * nth + tt
                    p_ps = pp.tile([P, A], F32, tag="p", name=f"pps_{r}_{t}")
                    if has_bias:
                        nc.tensor.matmul(p_ps[:, :], ones_row_bf[:], b_row[:],
                                         start=True, stop=False)
                    for k in range(KC):
                        nc.tensor.matmul(
                            p_ps[:, :],
                            XT[:, tt * KC + k, :],
                            w_sb[k][:],
                            start=(k == 0 and not has_bias),
                            stop=(k == KC - 1),
                        )
                    v = vp.tile([P, A], BF16, tag="v", name=f"v_{r}_{t}")
                    nc.scalar.activation(v[:], p_ps[:, :], AF.Tanh)
                    sc = scr.tile([P, A], BF16, tag="sc", name=f"sc_{r}_{t}")
                    nc.vector.scalar_tensor_tensor(
                        out=sc[:], in0=v[:], scalar=1.0, in1=u_b[:],
                        op0=ALU.mult, op1=ALU.mult,
                        accum_out=vu_row[:, t:t + 1],
                    )

            # ---- e = exp(vu): fp32 for alphas, bf16 for the weighted sum
            e_f = smp.tile([P, nt], F32, tag="ef")
            sums = smp.tile([P, 1], F32, tag="sums")
            nc.scalar.activation(e_f[:], vu_row[:], AF.Exp, accum_out=sums[:])
            e_bf = smp.tile([P, nt], BF16, tag="ebf")
            nc.vector.tensor_copy(e_bf[:], e_f[:])

            # ---- weighted sum: wsum[1, H] = sum_t e[:, t].T @ X_t
            wsum_ps = wp.tile([1, H], F32, tag="w")
            for t in range(nt):
                Xh = Xh_tiles[t // nth]
                tt = t % nth
                nc.tensor.matmul(
                    wsum_ps[:, :],
                    e_bf[:, t:t + 1],
                    Xh[:, tt * H:(tt + 1) * H],
                    start=(t == 0), stop=(t == nt - 1),
                )

            # ---- denom = sum(e) + 1e-10; rinv = 1/denom
            ssum_ps = tailp.tile([1, 1], F32, tag="tail")
            nc.tensor.matmul(ssum_ps[:, :], sums[:], ones_col_f, start=True, stop=True)
            denom = smp.tile([1, 1], F32, tag="denom")
            nc.vector.tensor_scalar_add(denom[:], ssum_ps[:, :], 1e-10)
            rinv = smp.tile([1, 1], F32, tag="rinv")
            nc.vector.reciprocal(rinv[:], denom[:])

            # ---- out row = wsum * rinv
            out_sb = smp.tile([1, H], F32, tag="outsb")
            nc.scalar.activation(out_sb[:], wsum_ps[:, :], AF.Copy, scale=rinv[:, :])
            nc.scalar.dma_start(out_d.ap()[r].rearrange("(o h) -> o h", o=1), out_sb[:])

            # ---- alphas row = e_f * rinv; element (p, t) -> s = nt*p + t
            bc_ps = tailp.tile([P, 1], F32, tag="tail")
            nc.tensor.matmul(bc_ps[:, :], ones_row_f[:], rinv[:], start=True, stop=True)
            r_b = smp.tile([P, 1], F32, tag="rb")
            nc.scalar.copy(r_b[:], bc_ps[:, :])
            a_sc = smp.tile([P, nt], F32, tag="asc")
            nc.vector.tensor_scalar_mul(a_sc[:], e_f[:], r_b[:])
            nc.scalar.dma_start(
                al_d.ap()[r].rearrange("(p t) -> p t", p=P), a_sc[:],
            )

    nc.compile()
    return nc


@functools.lru_cache(maxsize=2)
def _built(has_bias: bool):
    return build(R, S, has_bias)


def kernel(lstm_output, w_omega, b_omega, u_omega):
    lstm_output = np.ascontiguousarray(np.asarray(lstm_output, dtype=np.float32))
    w_omega = np.ascontiguousarray(np.asarray(w_omega, dtype=np.float32))
    b_omega = np.ascontiguousarray(np.asarray(b_omega, dtype=np.float32))
    u_omega = np.ascontiguousarray(np.asarray(u_omega, dtype=np.float32))

    has_bias = bool(np.any(b_omega != 0.0))
    nc = _built(has_bias)

    in_maps = []
    for c in range(N_CORES):
        m = {
            "x": lstm_output[c * R:(c + 1) * R],
            "w": w_omega,
            "u": u_omega,
        }
        if has_bias:
            m["b"] = b_omega
        in_maps.append(m)

    res = run_bass_kernel_spmd(nc, in_maps, core_ids=list(range(N_CORES)))
    out = np.concatenate([r["out"] for r in res.results], axis=0)
    alphas = np.concatenate([r["alphas"] for r in res.results], axis=0)
    return out, alphas
